# revision 13
# baseline (speedup 1.0000x reference)
"""GPT2 attention (B=2, S=2048, E=1024, H=16, interleaved QKV) on 8 trn2 NeuronCores.

Sharding: core c = 4*b + g handles batch b = c//4 and head group g = c%4
(heads 4g..4g+3): Megatron column-split of W_attn / row-split of W_proj,
data-parallel over batch. Host sums the 4 partial projection outputs per batch.

Design (throughput-oriented):
  - X^T is pre-transposed on the host; no PE transposes at all.
  - qk^T = W^T X^T (features on partitions); V computed directly in
    [token, dim] layout via x-stationary matmuls (no V transpose).
  - Scores S^T[sk,sq] per head with 64-deep contraction run 2-way
    concurrent on the two PE row-tiles (tile_position (0,0)/(64,0)),
    one head per half-array; both heads share one [128,2,512] score tile
    so every softmax exp call covers two blocks. Diagonal score blocks
    only compute (and exp) their valid (lower-trapezoid) width.
  - softmax exp is split between ACT (native Exp) and DVE (Schraudolph
    exponent-stuffing: round(x*a+b) as uint16, bitcast to f16) via a
    run-bounded load balancer. Large diagonal blocks (r=0,1) use exact
    ACT exp + a DVE 0/1 mask multiply; small ones (r=2,3) use a single
    fused DVE op whose mask-bias operand drives masked entries negative
    so the u16 convert saturates them to exactly 0.
  - PV appends 64 ones columns to V so PSUM rows 64..127 all hold the
    softmax denominators (replicated across partitions); the dens are
    staged to SBUF by two ACT copies, 1/den comes from an f16 bit-trick
    seed + one Newton step (3 DVE ops over both heads at once), and two
    DVE multiplies write normalized attnT — no PE broadcast matmuls.
  - the output projection is interleaved into the attention stream (one
    sq-chunk right after each PV burst, same 128x128 tile mode);
    evacuation is f16 alternating ACT/DVE into one [128,8,512] staging
    tile, flushed with a single DMA per sq chunk.
  - input DMAs are spread over the sync/scalar/gpsimd queues (each
    dma_start costs ~650ns of queue issue time) with wa[0]/xT[0] first.
"""
import numpy as np

import concourse.bass as bass
import concourse.bacc as bacc
import concourse.tile as tile
from concourse import mybir
from concourse.bass_utils import run_bass_kernel_spmd

F32 = mybir.dt.float32
F16 = mybir.dt.float16
I16 = mybir.dt.int16
U16 = mybir.dt.uint16

B, S, E, H = 2, 2048, 1024, 16
HD = E // H            # 64
HPC = 4                # heads per core
CW = HPC * 3 * HD      # 768: W_attn cols per core
CP = HPC * HD          # 256: W_proj rows per core
NK = E // 128          # 8 contraction chunks over E
NSQ = S // 512         # 4 sq chunks of 512
NSK = S // 128         # 16 sk chunks of 128

# Schraudolph exp: exp(x) ~= bitcast_f16(round(x * SCH_A + SCH_B)); the u16
# output convert saturates negatives to 0, so masked entries (biased by
# -60000 via the fused mask operand) become exactly +0.0
SCH_A = 1024.0 / float(np.log(2.0))
SCH_B = 15312.0  # 15360 - 48, f16-exact so the mask constant tiles match
MASKED = -60000.0
# f16 reciprocal seed: 1/d ~= bitcast_f16(RCP_K - bits_f16(d)), then one
# Newton step y1 = y0 * (2 - d*y0); max rel err ~3e-3
RCP_K = 30620.0

# dispatch cost model (ns) for the exp of an n-column tile
ACT_COST = lambda n: (n + 352) / 1.2
DVE_COST = lambda n: 0.52 * n + 300.0
DVE_NORM_EXTRA = 3900.0  # newton chain + normalization multiplies per (pair, J)

_cache = {}
_last_in_maps = None


def _build():
    from contextlib import ExitStack

    nc = bacc.Bacc("TRN2", target_bir_lowering=False, debug=False, num_devices=8)

    x_d = nc.dram_tensor("x", [NK, 128, S], F16, kind="ExternalInput").ap()
    wa_d = nc.dram_tensor("wa", [NK, 128, CW], F16, kind="ExternalInput").ap()
    ba_d = nc.dram_tensor("ba", [1, 128, 4], F32, kind="ExternalInput").ap()
    bv_d = nc.dram_tensor("bv", [1, 1, CP], F16, kind="ExternalInput").ap()
    wp_d = nc.dram_tensor("wp", [2, 128, E], F16, kind="ExternalInput").ap()
    out_d = nc.dram_tensor("out_t", [128, 8, S], F16, kind="ExternalOutput").ap()

    # diagonal-block masks in pair layout [h0 block | h1 block], reduced
    # coords g (col within block), keep where g >= p.
    # r=0,1 (large blocks): 0/1 multiply masks applied after an exact ACT exp.
    # r=2,3 (small blocks): fused bias masks for the DVE Schraudolph exp
    # (keep -> +SCH_B, masked -> -60000 so the u16 convert saturates to 0).
    gi = np.arange(512)[None, :]
    pi = np.arange(128)[:, None]
    mask01_d = {}
    maskB_d = {}
    for r in range(4):
        w = 512 - 128 * r
        keep = (gi < w) & (gi >= pi)
        if r < 2:
            half = np.zeros((128, 512), np.float16)
            half[keep] = 1.0
            mask01_d[r] = nc.inline_tensor(
                np.concatenate([half, half], axis=1), name=f"mask01_{r}"
            )
        else:
            half = np.full((128, 512), MASKED, np.float16)
            half[keep] = SCH_B
            maskB_d[r] = nc.inline_tensor(
                np.concatenate([half, half], axis=1), name=f"maskB{r}"
            )
    ones1_d = nc.inline_tensor(np.ones((1, 128), np.float16), name="ones1")

    Exp = mybir.ActivationFunctionType.Exp
    Ident = mybir.ActivationFunctionType.Identity
    Mult = mybir.AluOpType.mult
    Add = mybir.AluOpType.add

    with tile.TileContext(nc) as tc, ExitStack() as top:
        consts = top.enter_context(tc.tile_pool(name="consts", bufs=1))
        qk_pool = top.enter_context(tc.tile_pool(name="qkT", bufs=1))
        at_pool = top.enter_context(tc.tile_pool(name="attnT", bufs=1))
        wp_pool = top.enter_context(tc.tile_pool(name="wp", bufs=1))
        vb_pool = top.enter_context(tc.tile_pool(name="vb", bufs=1))
        xTp = top.enter_context(tc.tile_pool(name="xT", bufs=1))

        # ---- input DMAs -------------------------------------------------
        # each dma_start costs ~650ns of queue issue time, so spread the
        # loads over three queues and put the critical first inputs (wa[0],
        # xT[0]) at the head of their queues. xT[0] is split so its first
        # 512 columns (the first matmul's moving operand) land early.
        xT = [xTp.tile([128, S], F16, tag=f"xT{k}", name=f"xT{k}") for k in range(NK)]
        wa_t = consts.tile([128, NK, CW], F16)
        nc.sync.dma_start(out=xT[0][:, 0:512], in_=x_d[0][:, 0:512])
        nc.sync.dma_start(out=xT[0][:, 512:S], in_=x_d[0][:, 512:S])
        # gpsimd: wa0 wa1 xT2 wa2 wa3 xT5 wa4..wa7 so mid-k x chunks land
        # before the qkv k-loop reaches them
        gp_order = [("wa", 0), ("wa", 1), ("x", 2), ("wa", 2), ("wa", 3),
                    ("x", 5), ("wa", 4), ("wa", 5), ("wa", 6), ("wa", 7)]
        for kind, k in gp_order:
            if kind == "wa":
                nc.gpsimd.dma_start(out=wa_t[:, k, :], in_=wa_d[k])
            else:
                nc.gpsimd.dma_start(out=xT[k][:], in_=x_d[k])
        for k in (1, 3, 4):
            nc.scalar.dma_start(out=xT[k][:], in_=x_d[k])
        for k in (6, 7):
            nc.sync.dma_start(out=xT[k][:], in_=x_d[k])
        ba_t = consts.tile([128, 4], F32)
        nc.scalar.dma_start(out=ba_t[:], in_=ba_d[0])
        ones1_t = consts.tile([1, 128], F16)
        nc.scalar.dma_start(out=ones1_t[:], in_=ones1_d.ap())
        bv_t = consts.tile([1, CP], F16)
        nc.scalar.dma_start(out=bv_t[:], in_=bv_d[0])
        wp_t = wp_pool.tile([128, 2, E], F16)
        for cc in range(2):
            nc.sync.dma_start(out=wp_t[:, cc, :], in_=wp_d[cc])
        mask01_t = consts.tile([128, 2, 2, 512], F16)
        for r in range(2):
            nc.sync.dma_start(out=mask01_t[:, r, :, :], in_=mask01_d[r].ap())
        maskB_t = consts.tile([128, 2, 2, 512], F16)
        for r in range(2, 4):
            nc.sync.dma_start(out=maskB_t[:, r - 2, :, :], in_=maskB_d[r].ap())

        qkvT = [
            qk_pool.tile([128, S], F16, tag=f"qkT{cc}", name=f"qkT{cc}")
            for cc in range(4)
        ]
        attnT = [
            at_pool.tile([128, S], F16, tag=f"attnT{c}", name=f"attnT{c}")
            for c in range(2)
        ]
        # V with 64 ones columns appended: PSUM rows 64..127 of the PV
        # output all hold the softmax denominators (replicated), so
        # normalization is a copy + divide with no broadcast needed.
        vb4 = vb_pool.tile([128, NSK, HPC, 128], F16)
        nc.gpsimd.memset(vb4[:, :, :, 64:128], 1.0)

        # ---- phase 1: qk^T = W^T X^T, V = X Wv --------------------------
        with (
            tc.tile_pool(name="ps_mm", bufs=4, space="PSUM") as ps_mm,
            tc.tile_pool(name="ps_v", bufs=3, space="PSUM") as ps_v,
            tc.tile_pool(name="ps_b", bufs=1, space="PSUM") as ps_b,
        ):
            bvbc = consts.tile([128, HPC, 64], F32)

            for cc in range(4):
                if cc == 1:
                    # v-bias broadcast (32-row tile mode), tucked in after
                    # cc=0 so the kernel's first matmul is not gated on the
                    # small-constant DMAs
                    psb = ps_b.tile([128, HPC, 64], F32, tag="pvb", name="pvb")
                    nc.tensor.matmul(
                        psb[:], ones1_t[0:1, :], bv_t[0:1, :], start=True, stop=True
                    )
                    nc.vector.tensor_copy(bvbc[:], psb[:])
                pss = [
                    ps_mm.tile([128, 512], F32, tag="mm", name="mm_ps")
                    for _ in range(4)
                ]
                for k in range(NK):
                    lhs = wa_t[:, k, cc * 128 : (cc + 1) * 128]
                    for rc in range(4):
                        nc.tensor.matmul(
                            pss[rc][:],
                            lhs,
                            xT[k][:, rc * 512 : (rc + 1) * 512],
                            start=(k == 0),
                            stop=(k == NK - 1),
                        )
                for rc in range(4):
                    nc.scalar.activation(
                        qkvT[cc][:, rc * 512 : (rc + 1) * 512],
                        pss[rc][:],
                        Ident,
                        bias=ba_t[:, cc : cc + 1],
                        scale=0.125 if cc < 2 else 1.0,
                    )

            for i in range(NSK):
                psv = ps_v.tile([128, HPC, 64], F32, tag="pv", name="pv_ps")
                for k in range(NK):
                    nc.tensor.matmul(
                        psv[:],
                        xT[k][:, i * 128 : (i + 1) * 128],
                        wa_t[:, k, 512:768],
                        start=(k == 0),
                        stop=(k == NK - 1),
                    )
                nc.vector.tensor_add(vb4[:, i, :, 0:64], psv[:], bvbc[:])

        # ---- phase 2: per-head-pair attention ---------------------------
        # Both heads of a pair share one [128,2,512] score tile (one bank
        # per head), so every exp call covers two blocks. Diagonal blocks
        # compute + exp only their valid width. Off-diagonal blocks are
        # load-balanced between ACT Exp and DVE Schraudolph.
        with (
            tc.tile_pool(name="pp", bufs=18) as p_pool,
            tc.tile_pool(name="sm", bufs=2) as small,
            tc.tile_pool(name="ps_s", bufs=2, space="PSUM") as ps_s,
            tc.tile_pool(name="ps_pv", bufs=1, space="PSUM") as ps_pv,
            tc.tile_pool(name="ps_bc", bufs=2, space="PSUM") as ps_bc,
            tc.tile_pool(name="ob", bufs=2) as ob_pool,
        ):
            act_t = 0.0
            dve_t = 0.0
            last_eng = []

            def emit_proj(Jq):
                # projection for sq chunk Jq, interleaved into the attention
                # stream right after a PV burst (same 128x128 tile mode);
                # evacuations stage into one ob tile, flushed by one DMA
                ob = ob_pool.tile([128, 8, 512], F16, tag="ob", name="ob")
                for eo in range(8):
                    pp = ps_bc.tile([128, 512], F32, tag="bc", name="prj")
                    for cc in range(2):
                        nc.tensor.matmul(
                            pp[:],
                            wp_t[:, cc, eo * 128 : (eo + 1) * 128],
                            attnT[cc][:, Jq * 512 : (Jq + 1) * 512],
                            start=(cc == 0),
                            stop=(cc == 1),
                        )
                    if eo % 2 == 0:
                        nc.scalar.copy(ob[:, eo, :], pp[:])
                    else:
                        nc.vector.tensor_copy(ob[:, eo, :], pp[:])
                # single output DMA on the (otherwise idle) sync queue
                nc.sync.dma_start(
                    out=out_d[:, :, Jq * 512 : (Jq + 1) * 512], in_=ob[:]
                )

            for J in range(NSQ):
                for pr in range(2):
                    qT = qkvT[pr]
                    kT = qkvT[2 + pr]
                    nblk = 4 * J + 4
                    sq = bass.ts(J, 512)
                    # block order: diagonal r=0..3 first, then off-diagonal
                    order = [4 * J + r for r in range(4)] + list(range(4 * J))
                    pblks = {}
                    for i in order:
                        r = i - 4 * J
                        w = 512 if r < 0 else 512 - 128 * r
                        sqo = J * 512 + (0 if r < 0 else 128 * r)
                        stile = ps_s.tile([128, 2, 512], F32, tag="s", name="sps")
                        for hh in range(2):
                            o = hh * 64
                            nc.tensor.matmul(
                                stile[:, hh, 0:w],
                                kT[o : o + 64, i * 128 : (i + 1) * 128],
                                qT[o : o + 64, sqo : sqo + w],
                                start=True,
                                stop=True,
                                tile_position=(o, 0),
                            )
                        pt = p_pool.tile([128, 2, 512], U16, tag="p", name="p")
                        if r >= 0 and r < 2:
                            # large diagonal blocks: exact exp on ACT over the
                            # valid width, then a fast 0/1 mask multiply on DVE
                            # (full-width ops use merged 2D APs — 3D APs cost
                            # extra on full-width DVE ops)
                            act_t += ACT_COST(2 * w)
                            if r == 0:
                                nc.scalar.activation(
                                    pt[:].opt().bitcast(F16), stile[:].opt(), Exp
                                )
                                dve_t += DVE_COST(1024) * 0.5
                                nc.vector.tensor_mul(
                                    pt[:].opt().bitcast(F16),
                                    pt[:].opt().bitcast(F16),
                                    mask01_t[:, r, :, :].opt(),
                                )
                            else:
                                nc.scalar.activation(
                                    pt[:, :, 0:w].bitcast(F16),
                                    stile[:, :, 0:w],
                                    Exp,
                                )
                                dve_t += DVE_COST(2 * w) * 0.5
                                nc.vector.tensor_mul(
                                    pt[:, :, 0:w].bitcast(F16),
                                    pt[:, :, 0:w].bitcast(F16),
                                    mask01_t[:, r, :, 0:w],
                                )
                        elif r >= 2:
                            # small diagonal blocks: one fused DVE op
                            # (Schraudolph exp + mask bias, u16 saturation)
                            dve_t += DVE_COST(2 * w)
                            nc.vector.scalar_tensor_tensor(
                                out=pt[:, :, 0:w],
                                in0=stile[:, :, 0:w],
                                scalar=SCH_A,
                                in1=maskB_t[:, r - 2, :, 0:w],
                                op0=Mult,
                                op1=Add,
                            )
                        else:
                            ca, cd = ACT_COST(1024), DVE_COST(1024)
                            use_act = act_t + ca <= dve_t + cd
                            # bound same-engine runs at 2 so the two
                            # in-flight score tiles are never serialized
                            # behind a single engine
                            if len(last_eng) >= 2 and last_eng[-1] == last_eng[-2]:
                                use_act = not last_eng[-1]
                            last_eng.append(use_act)
                            if use_act:
                                act_t += ca
                                nc.scalar.activation(
                                    pt[:].opt().bitcast(F16), stile[:].opt(), Exp
                                )
                            else:
                                dve_t += cd
                                nc.vector.tensor_scalar(
                                    pt[:].opt(), stile[:].opt(), SCH_A, SCH_B,
                                    Mult, Add,
                                )
                        pblks[i] = pt

                    pvp = ps_pv.tile([128, 2, 512], F32, tag="pv", name="pvp")
                    dn = small.tile([128, 1024], F16, tag="dn", name="dn")
                    y0 = small.tile([128, 1024], I16, tag="y0", name="y0")
                    t1 = small.tile([128, 1024], F16, tag="t1", name="t1")
                    for hh in range(2):
                        for n, i in enumerate(order):
                            r = i - 4 * J
                            w = 512 if r < 0 else 512 - 128 * r
                            co = 0 if r < 0 else 128 * r
                            nc.tensor.matmul(
                                pvp[:, hh, co : co + w],
                                vb4[:, i, 2 * pr + hh, :],
                                pblks[i][:, hh, 0:w].bitcast(F16),
                                start=(n == 0),
                                stop=(n == nblk - 1),
                            )
                        # rows 64..127 of pvp all hold this head's softmax
                        # denominators (replicated); stage to SBUF f16 on
                        # ACT (Copy shares the exp act table)
                        act_t += ACT_COST(512)
                        nc.scalar.copy(
                            dn[0:64, hh * 512 : (hh + 1) * 512],
                            pvp[64:128, hh, :],
                        )
                    # 1/den: f16 bit-trick seed + one Newton step, both heads
                    # in one pass; then per-head multiplies into attnT
                    nc.vector.tensor_scalar(
                        y0[0:64, :], dn[0:64, :].bitcast(I16),
                        -1.0, RCP_K, Mult, Add,
                    )
                    nc.vector.scalar_tensor_tensor(
                        out=t1[0:64, :], in0=dn[0:64, :], scalar=-1.0,
                        in1=y0[0:64, :].bitcast(F16), op0=Mult, op1=Mult,
                    )
                    nc.vector.scalar_tensor_tensor(
                        out=t1[0:64, :], in0=t1[0:64, :], scalar=2.0,
                        in1=y0[0:64, :].bitcast(F16), op0=Add, op1=Mult,
                    )
                    for hh in range(2):
                        nc.vector.tensor_mul(
                            attnT[pr][hh * 64 : hh * 64 + 64, sq],
                            pvp[0:64, hh, :],
                            t1[0:64, hh * 512 : (hh + 1) * 512],
                        )
                    dve_t += DVE_NORM_EXTRA

                    if pr == 0 and J >= 1:
                        emit_proj(J - 1)
            emit_proj(3)

    nc.compile()
    return nc


def _col_perm(g):
    """Per-core W_attn column permutation: [q0..q3 | k0..k3 | v0..v3]."""
    cols = []
    for t in range(3):          # q, k, v
        for h in range(HPC):
            base = (4 * g + h) * 3 * HD + t * HD
            cols.append(np.arange(base, base + HD))
    return np.concatenate(cols)


def kernel(hidden_states, W_attn, b_attn, W_proj, b_proj):
    hidden_states = np.asarray(hidden_states, np.float32)
    W_attn = np.asarray(W_attn, np.float32)
    b_attn = np.asarray(b_attn, np.float32)
    W_proj = np.asarray(W_proj, np.float32)
    b_proj = np.asarray(b_proj, np.float32)

    if "nc" not in _cache:
        _cache["nc"] = _build()
    nc = _cache["nc"]

    # q columns (first 256 of the permuted layout) have scale 1/8 folded into
    # the PSUM->SBUF copy; bias is added after the scale, so pre-scale it.
    bias_scale = np.ones(2 * CP, np.float32)
    bias_scale[:CP] = 0.125

    in_maps = []
    for c in range(8):
        b, g = divmod(c, 4)
        perm = _col_perm(g)
        wa = np.ascontiguousarray(W_attn[:, perm])
        ba = np.ascontiguousarray(
            (b_attn[perm][: 2 * CP] * bias_scale).astype(np.float32).reshape(4, 128).T
        )
        bv = b_attn[perm][2 * CP :].astype(np.float16)
        wp = np.ascontiguousarray(W_proj[g * CP : (g + 1) * CP, :])
        xT = np.ascontiguousarray(hidden_states[b].T).astype(np.float16)
        in_maps.append(
            {
                "x": xT.reshape(NK, 128, S),
                "wa": wa.astype(np.float16).reshape(NK, 128, CW),
                "ba": ba.reshape(1, 128, 4),
                "bv": bv.reshape(1, 1, CP),
                "wp": wp.astype(np.float16).reshape(2, 128, E),
            }
        )

    global _last_in_maps
    _last_in_maps = in_maps
    res = run_bass_kernel_spmd(nc, in_maps, list(range(8)))

    out = np.zeros((B, S, E), np.float32)
    for c in range(8):
        b = c // 4
        # out_t is [128, 8, S]; output feature e = eo*128 + p
        ot = res.results[c]["out_t"].transpose(1, 0, 2).reshape(E, S)
        out[b] += ot.astype(np.float32).T
    out += b_proj
    return out


# revision 23
# speedup vs baseline: 1.0654x; 1.0654x over previous
"""GPT2 attention (B=2, S=2048, E=1024, H=16, interleaved QKV) on 8 trn2 NeuronCores.

Sharding: core c = 4*b + g handles batch b = c//4 and head group g = c%4
(heads 4g..4g+3): Megatron column-split of W_attn / row-split of W_proj,
data-parallel over batch. Host sums the 4 partial projection outputs per batch.

Design (throughput-oriented):
  - X^T is pre-transposed on the host; no PE transposes at all.
  - qk^T = W^T X^T (features on partitions); V computed directly in
    [token, dim] layout via x-stationary matmuls (no V transpose).
  - The QKV projection is NOT a separate phase: only pair-0's first
    q/k chunks and first V blocks run up front (~6us); the rest is cut
    into small filler units (one PSUM-bank lifetime each) that are
    pumped into the attention score streams wherever the PE would
    otherwise idle waiting for softmax exps. Groups run pair-major
    ((pr0,J0..3) then (pr1,J0..3)) with a deadline scheduler forcing
    each unit to complete before the group that reads it.
  - Scores S^T[sk,sq] per head with 64-deep contraction run 2-way
    concurrent on the two PE row-tiles (tile_position (0,0)/(64,0)),
    one head per half-array; both heads share one [128,2,512] score tile
    so every softmax exp call covers two blocks. Diagonal score blocks
    only compute (and exp) their valid (lower-trapezoid) width.
  - softmax exp is split between ACT (native Exp) and DVE (Schraudolph
    exponent-stuffing: round(x*a+b) as uint16, bitcast to f16) via a
    run-bounded load balancer. Large diagonal blocks (r=0,1) use exact
    ACT exp + a DVE 0/1 mask multiply; small ones (r=2,3) use a single
    fused DVE op whose mask-bias operand drives masked entries negative
    so the u16 convert saturates them to exactly 0.
  - PV appends 64 ones columns to V so PSUM rows 64..127 all hold the
    softmax denominators (replicated across partitions); the dens are
    staged to SBUF by two ACT copies, 1/den comes from an f16 bit-trick
    seed + one Newton step (3 DVE ops over both heads at once), and two
    DVE multiplies write normalized attnT — no PE broadcast matmuls.
  - the output projection for sq chunk J runs right after (pr1, J)'s
    normalization; evacuations are f16 alternating ACT/DVE into one
    [128,8,512] staging tile, flushed with two half DMAs per chunk.
  - input DMAs are spread over the sync/scalar/gpsimd queues (each
    dma_start costs ~650ns of queue issue time) with wa[0]/xT[0] first.
"""
import numpy as np

import concourse.bass as bass
import concourse.bacc as bacc
import concourse.tile as tile
from concourse import mybir
from concourse.bass_utils import run_bass_kernel_spmd

F32 = mybir.dt.float32
F16 = mybir.dt.float16
I16 = mybir.dt.int16
U16 = mybir.dt.uint16

B, S, E, H = 2, 2048, 1024, 16
HD = E // H            # 64
HPC = 4                # heads per core
CW = HPC * 3 * HD      # 768: W_attn cols per core
CP = HPC * HD          # 256: W_proj rows per core
NK = E // 128          # 8 contraction chunks over E
NSQ = S // 512         # 4 sq chunks of 512
NSK = S // 128         # 16 sk chunks of 128

# Schraudolph exp: exp(x) ~= bitcast_f16(round(x * SCH_A + SCH_B)); the u16
# output convert saturates negatives to 0, so masked entries (biased by
# -60000 via the fused mask operand) become exactly +0.0
SCH_A = 1024.0 / float(np.log(2.0))
SCH_B = 15312.0  # 15360 - 48, f16-exact so the mask constant tiles match
MASKED = -60000.0
# f16 reciprocal seed: 1/d ~= bitcast_f16(RCP_K - bits_f16(d)), then one
# Newton step y1 = y0 * (2 - d*y0); max rel err ~3e-3
RCP_K = 30620.0

# dispatch cost model (ns) for the exp of an n-column tile
ACT_COST = lambda n: (n + 352) / 1.2
DVE_COST = lambda n: 0.52 * n + 300.0
DVE_NORM_EXTRA = 3900.0  # newton chain + normalization multiplies per (pair, J)

_cache = {}
_last_in_maps = None


def _build():
    from contextlib import ExitStack

    nc = bacc.Bacc("TRN2", target_bir_lowering=False, debug=False, num_devices=8)

    x_d = nc.dram_tensor("x", [NK, 128, S], F16, kind="ExternalInput").ap()
    wa_d = nc.dram_tensor("wa", [NK, 128, CW], F16, kind="ExternalInput").ap()
    ba_d = nc.dram_tensor("ba", [1, 128, 4], F32, kind="ExternalInput").ap()
    bv_d = nc.dram_tensor("bv", [1, 1, CP], F16, kind="ExternalInput").ap()
    wp_d = nc.dram_tensor("wp", [2, 128, E], F16, kind="ExternalInput").ap()
    out_d = nc.dram_tensor("out_t", [128, 8, S], F16, kind="ExternalOutput").ap()

    gi = np.arange(512)[None, :]
    pi = np.arange(128)[:, None]
    mask01_d = {}
    maskB_d = {}
    for r in range(4):
        w = 512 - 128 * r
        keep = (gi < w) & (gi >= pi)
        if r < 2:
            half = np.zeros((128, 512), np.float16)
            half[keep] = 1.0
            mask01_d[r] = nc.inline_tensor(
                np.concatenate([half, half], axis=1), name=f"mask01_{r}"
            )
        else:
            half = np.full((128, 512), MASKED, np.float16)
            half[keep] = SCH_B
            maskB_d[r] = nc.inline_tensor(
                np.concatenate([half, half], axis=1), name=f"maskB{r}"
            )
    ones1_d = nc.inline_tensor(np.ones((1, 128), np.float16), name="ones1")

    Exp = mybir.ActivationFunctionType.Exp
    Ident = mybir.ActivationFunctionType.Identity
    Mult = mybir.AluOpType.mult
    Add = mybir.AluOpType.add

    with tile.TileContext(nc) as tc, ExitStack() as top:
        consts = top.enter_context(tc.tile_pool(name="consts", bufs=1))
        qk_pool = top.enter_context(tc.tile_pool(name="qkT", bufs=1))
        at_pool = top.enter_context(tc.tile_pool(name="attnT", bufs=1))
        wp_pool = top.enter_context(tc.tile_pool(name="wp", bufs=1))
        vb_pool = top.enter_context(tc.tile_pool(name="vb", bufs=1))
        xTp = top.enter_context(tc.tile_pool(name="xT", bufs=1))

        # ---- input DMAs -------------------------------------------------
        # spread over four queues (sync/gpsimd/scalar/vector), with wa[0],
        # xT[0], and the small constants first so the phase-A k-major sweep
        # and the v-bias broadcast are never DMA-gated for long
        xT = [xTp.tile([128, S], F16, tag=f"xT{k}", name=f"xT{k}") for k in range(NK)]
        wa_t = consts.tile([128, NK, CW], F16)
        # V with 64 ones columns appended: PSUM rows 64..127 of the PV
        # output all hold the softmax denominators (replicated). NOTE: the
        # ones memset must stay on gpsimd — the DVE InstMemset mis-lowers
        # this strided 4D region (wrong stride/count merge) and corrupts
        # the denominators.
        vb4 = vb_pool.tile([128, NSK, HPC, 128], F16)
        bvbc = consts.tile([128, HPC, 64], F32)

        nc.sync.dma_start(out=xT[0][:, 0:512], in_=x_d[0][:, 0:512])
        nc.sync.dma_start(out=xT[0][:, 512:S], in_=x_d[0][:, 512:S])
        for k in (2, 5, 6, 7):
            nc.sync.dma_start(out=xT[k][:], in_=x_d[k])
        for k in range(NK):
            nc.gpsimd.dma_start(out=wa_t[:, k, :], in_=wa_d[k])
        nc.gpsimd.memset(vb4[:, 0:4, :, 64:128], 1.0)
        ba_t = consts.tile([128, 4], F32)
        nc.scalar.dma_start(out=ba_t[:], in_=ba_d[0])
        ones1_t = consts.tile([1, 128], F16)
        nc.scalar.dma_start(out=ones1_t[:], in_=ones1_d.ap())
        bv_t = consts.tile([1, CP], F16)
        nc.scalar.dma_start(out=bv_t[:], in_=bv_d[0])
        for k in (1, 3, 4):
            nc.scalar.dma_start(out=xT[k][:], in_=x_d[k])
        wp_t = wp_pool.tile([128, 2, E], F16)
        for cc in range(2):
            nc.sync.dma_start(out=wp_t[:, cc, :], in_=wp_d[cc])
        mask01_t = consts.tile([128, 2, 2, 512], F16)
        for r in range(2):
            nc.scalar.dma_start(out=mask01_t[:, r, :, :], in_=mask01_d[r].ap())
        maskB_t = consts.tile([128, 2, 2, 512], F16)
        for r in range(2, 4):
            nc.gpsimd.dma_start(out=maskB_t[:, r - 2, :, :], in_=maskB_d[r].ap())
        nc.gpsimd.memset(vb4[:, 4:NSK, :, 64:128], 1.0)

        qkvT = [
            qk_pool.tile([128, S], F16, tag=f"qkT{cc}", name=f"qkT{cc}")
            for cc in range(4)
        ]
        attnT = [
            at_pool.tile([128, S], F16, tag=f"attnT{c}", name=f"attnT{c}")
            for c in range(2)
        ]

        with (
            tc.tile_pool(name="pp", bufs=18) as p_pool,
            tc.tile_pool(name="sm", bufs=2) as small,
            tc.tile_pool(name="ps_s", bufs=2, space="PSUM") as ps_s,
            tc.tile_pool(name="ps_pv", bufs=1, space="PSUM") as ps_pv,
            tc.tile_pool(name="ps_aux", bufs=2, space="PSUM") as ps_aux,
            tc.tile_pool(name="ob", bufs=2) as ob_pool,
        ):
            # ---- filler units: QKV/V work cut into PSUM-bank-lifetime
            # generators, pumped into attention-stream PE bubbles --------
            def qk_unit_gen(cc, rc):
                ps = ps_aux.tile([128, 512], F32, tag="aux", name="fq")
                for k in range(NK):
                    nc.tensor.matmul(
                        ps[:],
                        wa_t[:, k, cc * 128 : (cc + 1) * 128],
                        xT[k][:, rc * 512 : (rc + 1) * 512],
                        start=(k == 0),
                        stop=(k == NK - 1),
                    )
                    if k % 2 == 1 and k < NK - 1:
                        yield
                nc.scalar.activation(
                    qkvT[cc][:, rc * 512 : (rc + 1) * 512],
                    ps[:],
                    Ident,
                    bias=ba_t[:, cc : cc + 1],
                    scale=0.125 if cc < 2 else 1.0,
                )

            def v_unit_gen(pr, i):
                ps = ps_aux.tile([128, 512], F32, tag="aux", name="fv")
                for k in range(NK):
                    nc.tensor.matmul(
                        ps[:, 0:128],
                        xT[k][:, i * 128 : (i + 1) * 128],
                        wa_t[:, k, 512 + pr * 128 : 640 + pr * 128],
                        start=(k == 0),
                        stop=(k == NK - 1),
                    )
                for h in range(2):
                    nc.vector.tensor_add(
                        vb4[:, i, 2 * pr + h, 0:64],
                        ps[:, h * 64 : h * 64 + 64],
                        bvbc[:, 2 * pr + h, :],
                    )
                yield

            # work queue: (deadline_group, generator, started); deadlines are
            # one group EARLY (evacuations queue behind a whole group of
            # exps, so a unit finished exactly at its reader's boundary
            # would stall the reader ~5us on the evac)
            work = []
            for g in (1, 2, 3):
                d = max(1, g - 1)
                work.append([d, qk_unit_gen(0, g), False])
                work.append([d, qk_unit_gen(2, g), False])
                for i in range(4 * g, 4 * g + 4):
                    work.append([d, v_unit_gen(0, i), False])
            for g in (4, 5, 6, 7):
                J = g - 4
                work.append([g - 1, qk_unit_gen(1, J), False])
                work.append([g - 1, qk_unit_gen(3, J), False])
                for i in range(4 * J, 4 * J + 4):
                    work.append([g - 1, v_unit_gen(1, i), False])

            def pump(n):
                for _ in range(n):
                    if not work:
                        return
                    ent = work[0]
                    ent[2] = True
                    try:
                        next(ent[1])
                    except StopIteration:
                        work.pop(0)

            def finish_inflight():
                # run a suspended unit to completion so its PSUM ring slot
                # is safe to rotate past (emit_proj allocates from the ring)
                if work and work[0][2]:
                    for _ in work[0][1]:
                        pass
                    work.pop(0)

            def drain_until(G):
                while work and work[0][0] <= G:
                    for _ in work[0][1]:
                        pass
                    work.pop(0)

            # ---- phase A: pair-0 prerequisites up front ----------------
            for _ in qk_unit_gen(0, 0):
                pass
            psb = ps_aux.tile([128, 512], F32, tag="aux", name="pvb")
            nc.tensor.matmul(
                psb[:, 0:CP], ones1_t[0:1, :], bv_t[0:1, :], start=True, stop=True
            )
            nc.vector.tensor_copy(bvbc[:].opt(), psb[:, 0:CP])
            for _ in qk_unit_gen(2, 0):
                pass
            for i in range(4):
                for _ in v_unit_gen(0, i):
                    pass

            act_t = 0.0
            dve_t = 0.0
            last_eng = []

            def emit_proj(Jq):
                ob = ob_pool.tile([128, 8, 512], F16, tag="ob", name="ob")
                for eo in range(8):
                    pp = ps_aux.tile([128, 512], F32, tag="aux", name="prj")
                    for cc in range(2):
                        nc.tensor.matmul(
                            pp[:],
                            wp_t[:, cc, eo * 128 : (eo + 1) * 128],
                            attnT[cc][:, Jq * 512 : (Jq + 1) * 512],
                            start=(cc == 0),
                            stop=(cc == 1),
                        )
                    if eo % 2 == 0:
                        nc.scalar.copy(ob[:, eo, :], pp[:])
                    else:
                        nc.vector.tensor_copy(ob[:, eo, :], pp[:])
                    if eo == 3:
                        nc.sync.dma_start(
                            out=out_d[:, 0:4, Jq * 512 : (Jq + 1) * 512],
                            in_=ob[:, 0:4, :],
                        )
                nc.sync.dma_start(
                    out=out_d[:, 4:8, Jq * 512 : (Jq + 1) * 512],
                    in_=ob[:, 4:8, :],
                )

            pending = []
            for G in range(8):
                pr, J = G // 4, G % 4
                drain_until(G)
                qT = qkvT[pr]
                kT = qkvT[2 + pr]
                nblk = 4 * J + 4
                sq = bass.ts(J, 512)
                order = [4 * J + r for r in range(4)] + list(range(4 * J))
                pblks = {}
                nissued = 0
                for i in order:
                    r = i - 4 * J
                    w = 512 if r < 0 else 512 - 128 * r
                    sqo = J * 512 + (0 if r < 0 else 128 * r)
                    stile = ps_s.tile([128, 2, 512], F32, tag="s", name="sps")
                    for hh in range(2):
                        o = hh * 64
                        nc.tensor.matmul(
                            stile[:, hh, 0:w],
                            kT[o : o + 64, i * 128 : (i + 1) * 128],
                            qT[o : o + 64, sqo : sqo + w],
                            start=True,
                            stop=True,
                            tile_position=(o, 0),
                        )
                    pt = p_pool.tile([128, 2, 512], U16, tag="p", name="p")
                    if r >= 0 and r < 2:
                        act_t += ACT_COST(2 * w)
                        if r == 0:
                            nc.scalar.activation(
                                pt[:].opt().bitcast(F16), stile[:].opt(), Exp
                            )
                            dve_t += DVE_COST(1024) * 0.5
                            nc.vector.tensor_mul(
                                pt[:].opt().bitcast(F16),
                                pt[:].opt().bitcast(F16),
                                mask01_t[:, r, :, :].opt(),
                            )
                        else:
                            nc.scalar.activation(
                                pt[:, :, 0:w].bitcast(F16), stile[:, :, 0:w], Exp
                            )
                            dve_t += DVE_COST(2 * w) * 0.5
                            nc.vector.tensor_mul(
                                pt[:, :, 0:w].bitcast(F16),
                                pt[:, :, 0:w].bitcast(F16),
                                mask01_t[:, r, :, 0:w],
                            )
                    elif r >= 2:
                        dve_t += DVE_COST(2 * w)
                        nc.vector.scalar_tensor_tensor(
                            out=pt[:, :, 0:w],
                            in0=stile[:, :, 0:w],
                            scalar=SCH_A,
                            in1=maskB_t[:, r - 2, :, 0:w],
                            op0=Mult,
                            op1=Add,
                        )
                    else:
                        ca, cd = ACT_COST(1024), DVE_COST(1024)
                        use_act = act_t + ca <= dve_t + cd
                        if len(last_eng) >= 2 and last_eng[-1] == last_eng[-2]:
                            use_act = not last_eng[-1]
                        last_eng.append(use_act)
                        if use_act:
                            act_t += ca
                            nc.scalar.activation(
                                pt[:].opt().bitcast(F16), stile[:].opt(), Exp
                            )
                        else:
                            dve_t += cd
                            nc.vector.tensor_scalar(
                                pt[:].opt(), stile[:].opt(), SCH_A, SCH_B,
                                Mult, Add,
                            )
                    pblks[i] = pt
                    nissued += 1
                    pump(1)
                    # the previous chunk's projection lands a few blocks
                    # into this stream, after its attnT inputs are written
                    if nissued == 3 and pending:
                        finish_inflight()
                        for fn in pending:
                            fn()
                        pending = []
                if pending:
                    finish_inflight()
                    for fn in pending:
                        fn()
                    pending = []

                last = G == 7
                pvp = ps_pv.tile([128, 2, 512], F32, tag="pv", name="pvp")
                dn = small.tile([128, 1024], F16, tag="dn", name="dn")
                y0 = small.tile([128, 1024], I16, tag="y0", name="y0")
                t1 = small.tile([128, 1024], F16, tag="t1", name="t1")
                for hh in range(2):
                    hs = slice(hh * 512, (hh + 1) * 512)
                    for n, i in enumerate(order):
                        r = i - 4 * J
                        w = 512 if r < 0 else 512 - 128 * r
                        co = 0 if r < 0 else 128 * r
                        nc.tensor.matmul(
                            pvp[:, hh, co : co + w],
                            vb4[:, i, 2 * pr + hh, :],
                            pblks[i][:, hh, 0:w].bitcast(F16),
                            start=(n == 0),
                            stop=(n == nblk - 1),
                        )
                    act_t += ACT_COST(512)
                    nc.scalar.copy(dn[0:64, hs], pvp[64:128, hh, :])
                    if last:
                        # tail: run the whole reciprocal + normalize chain
                        # per head so head 0's chain hides under head 1's
                        # PV burst
                        nc.vector.tensor_scalar(
                            y0[0:64, hs], dn[0:64, hs].bitcast(I16),
                            -1.0, RCP_K, Mult, Add,
                        )
                        nc.vector.scalar_tensor_tensor(
                            out=t1[0:64, hs], in0=dn[0:64, hs], scalar=-1.0,
                            in1=y0[0:64, hs].bitcast(F16), op0=Mult, op1=Mult,
                        )
                        nc.vector.scalar_tensor_tensor(
                            out=t1[0:64, hs], in0=t1[0:64, hs], scalar=2.0,
                            in1=y0[0:64, hs].bitcast(F16), op0=Add, op1=Mult,
                        )
                        nc.vector.tensor_mul(
                            attnT[pr][hh * 64 : hh * 64 + 64, sq],
                            pvp[0:64, hh, :],
                            t1[0:64, hs],
                        )
                if not last:
                    nc.vector.tensor_scalar(
                        y0[0:64, :], dn[0:64, :].bitcast(I16),
                        -1.0, RCP_K, Mult, Add,
                    )
                    nc.vector.scalar_tensor_tensor(
                        out=t1[0:64, :], in0=dn[0:64, :], scalar=-1.0,
                        in1=y0[0:64, :].bitcast(F16), op0=Mult, op1=Mult,
                    )
                    nc.vector.scalar_tensor_tensor(
                        out=t1[0:64, :], in0=t1[0:64, :], scalar=2.0,
                        in1=y0[0:64, :].bitcast(F16), op0=Add, op1=Mult,
                    )
                    for hh in range(2):
                        nc.vector.tensor_mul(
                            attnT[pr][hh * 64 : hh * 64 + 64, sq],
                            pvp[0:64, hh, :],
                            t1[0:64, hh * 512 : (hh + 1) * 512],
                        )
                dve_t += DVE_NORM_EXTRA

                if pr == 1:
                    if last:
                        emit_proj(J)
                    else:
                        pending.append(lambda Jq=J: emit_proj(Jq))

    nc.compile()
    return nc


def _col_perm(g):
    """Per-core W_attn column permutation: [q0..q3 | k0..k3 | v0..v3]."""
    cols = []
    for t in range(3):          # q, k, v
        for h in range(HPC):
            base = (4 * g + h) * 3 * HD + t * HD
            cols.append(np.arange(base, base + HD))
    return np.concatenate(cols)


def kernel(hidden_states, W_attn, b_attn, W_proj, b_proj):
    hidden_states = np.asarray(hidden_states, np.float32)
    W_attn = np.asarray(W_attn, np.float32)
    b_attn = np.asarray(b_attn, np.float32)
    W_proj = np.asarray(W_proj, np.float32)
    b_proj = np.asarray(b_proj, np.float32)

    if "nc" not in _cache:
        _cache["nc"] = _build()
    nc = _cache["nc"]

    # q columns (first 256 of the permuted layout) have scale 1/8 folded into
    # the PSUM->SBUF copy; bias is added after the scale, so pre-scale it.
    bias_scale = np.ones(2 * CP, np.float32)
    bias_scale[:CP] = 0.125

    in_maps = []
    for c in range(8):
        b, g = divmod(c, 4)
        perm = _col_perm(g)
        wa = np.ascontiguousarray(W_attn[:, perm])
        ba = np.ascontiguousarray(
            (b_attn[perm][: 2 * CP] * bias_scale).astype(np.float32).reshape(4, 128).T
        )
        bv = b_attn[perm][2 * CP :].astype(np.float16)
        wp = np.ascontiguousarray(W_proj[g * CP : (g + 1) * CP, :])
        xT = np.ascontiguousarray(hidden_states[b].T).astype(np.float16)
        in_maps.append(
            {
                "x": xT.reshape(NK, 128, S),
                "wa": wa.astype(np.float16).reshape(NK, 128, CW),
                "ba": ba.reshape(1, 128, 4),
                "bv": bv.reshape(1, 1, CP),
                "wp": wp.astype(np.float16).reshape(2, 128, E),
            }
        )

    global _last_in_maps
    _last_in_maps = in_maps
    res = run_bass_kernel_spmd(nc, in_maps, list(range(8)))

    out = np.zeros((B, S, E), np.float32)
    for c in range(8):
        b = c // 4
        # out_t is [128, 8, S]; output feature e = eo*128 + p
        ot = res.results[c]["out_t"].transpose(1, 0, 2).reshape(E, S)
        out[b] += ot.astype(np.float32).T
    out += b_proj
    return out


# revision 25
# speedup vs baseline: 1.1106x; 1.0424x over previous
"""GPT2 attention (B=2, S=2048, E=1024, H=16, interleaved QKV) on 8 trn2 NeuronCores.

Sharding: core c = 4*b + g handles batch b = c//4 and head group g = c%4
(heads 4g..4g+3): Megatron column-split of W_attn / row-split of W_proj,
data-parallel over batch. Host sums the 4 partial projection outputs per batch.

Design (throughput-oriented):
  - X^T is pre-transposed on the host; no PE transposes at all.
  - qk^T = W^T X^T (features on partitions); V computed directly in
    [token, dim] layout via x-stationary matmuls (no V transpose).
  - The QKV projection is NOT a separate phase: only pair-0's first
    q/k chunks and first V blocks run up front (~6us); the rest is cut
    into small filler units (one PSUM-bank lifetime each) that are
    pumped into the attention score streams wherever the PE would
    otherwise idle waiting for softmax exps. Groups run pair-major
    ((pr0,J0..3) then (pr1,J0..3)) with a deadline scheduler forcing
    each unit to complete before the group that reads it.
  - Scores S^T[sk,sq] per head with 64-deep contraction run 2-way
    concurrent on the two PE row-tiles (tile_position (0,0)/(64,0)),
    one head per half-array; both heads share one [128,2,512] score tile
    so every softmax exp call covers two blocks. Diagonal score blocks
    only compute (and exp) their valid (lower-trapezoid) width.
  - softmax exp is split between ACT (native Exp) and DVE (Schraudolph
    exponent-stuffing: round(x*a+b) as uint16, bitcast to f16) via a
    run-bounded load balancer. Large diagonal blocks (r=0,1) use exact
    ACT exp + a DVE 0/1 mask multiply; small ones (r=2,3) use a single
    fused DVE op whose mask-bias operand drives masked entries negative
    so the u16 convert saturates them to exactly 0.
  - PV appends 64 ones columns to V so PSUM rows 64..127 all hold the
    softmax denominators (replicated across partitions); the dens are
    staged to SBUF by two ACT copies, 1/den comes from an f16 bit-trick
    seed + one Newton step (3 DVE ops over both heads at once), and two
    DVE multiplies write normalized attnT — no PE broadcast matmuls.
  - the output projection for sq chunk J runs right after (pr1, J)'s
    normalization; evacuations are f16 alternating ACT/DVE into one
    [128,8,512] staging tile, flushed with two half DMAs per chunk.
  - input DMAs are spread over the sync/scalar/gpsimd queues (each
    dma_start costs ~650ns of queue issue time) with wa[0]/xT[0] first.
"""
import numpy as np

import concourse.bass as bass
import concourse.bacc as bacc
import concourse.tile as tile
from concourse import mybir
from concourse.bass_utils import run_bass_kernel_spmd

F32 = mybir.dt.float32
F16 = mybir.dt.float16
I16 = mybir.dt.int16
U16 = mybir.dt.uint16

B, S, E, H = 2, 2048, 1024, 16
HD = E // H            # 64
HPC = 4                # heads per core
CW = HPC * 3 * HD      # 768: W_attn cols per core
CP = HPC * HD          # 256: W_proj rows per core
NK = E // 128          # 8 contraction chunks over E
NSQ = S // 512         # 4 sq chunks of 512
NSK = S // 128         # 16 sk chunks of 128

# Schraudolph exp: exp(x) ~= bitcast_f16(round(x * SCH_A + SCH_B)); the u16
# output convert saturates negatives to 0, so masked entries (biased by
# -60000 via the fused mask operand) become exactly +0.0
SCH_A = 1024.0 / float(np.log(2.0))
SCH_B = 15312.0  # 15360 - 48, f16-exact so the mask constant tiles match
MASKED = -60000.0
# f16 reciprocal seed: 1/d ~= bitcast_f16(RCP_K - bits_f16(d)), then one
# Newton step y1 = y0 * (2 - d*y0); max rel err ~3e-3
RCP_K = 30620.0

# dispatch cost model (ns) for the exp of an n-column tile
ACT_COST = lambda n: (n + 352) / 1.2
DVE_COST = lambda n: 0.85 * n + 300.0  # recalibrated from HW traces
DVE_NORM_EXTRA = 3900.0  # newton chain + normalization multiplies per (pair, J)

_cache = {}
_last_in_maps = None


def _build():
    from contextlib import ExitStack

    nc = bacc.Bacc("TRN2", target_bir_lowering=False, debug=False, num_devices=8)

    x_d = nc.dram_tensor("x", [NK, 128, S], F16, kind="ExternalInput").ap()
    wa_d = nc.dram_tensor("wa", [NK, 128, CW], F16, kind="ExternalInput").ap()
    ba_d = nc.dram_tensor("ba", [1, 128, 4], F32, kind="ExternalInput").ap()
    bv_d = nc.dram_tensor("bv", [1, 1, CP], F16, kind="ExternalInput").ap()
    wp_d = nc.dram_tensor("wp", [2, 128, E], F16, kind="ExternalInput").ap()
    out_d = nc.dram_tensor("out_t", [128, 8, S], F16, kind="ExternalOutput").ap()

    gi = np.arange(512)[None, :]
    pi = np.arange(128)[:, None]
    mask01_d = {}
    maskB_d = {}
    for r in range(4):
        w = 512 - 128 * r
        keep = (gi < w) & (gi >= pi)
        if r < 2:
            half = np.zeros((128, 512), np.float16)
            half[keep] = 1.0
            mask01_d[r] = nc.inline_tensor(
                np.concatenate([half, half], axis=1), name=f"mask01_{r}"
            )
        else:
            half = np.full((128, 512), MASKED, np.float16)
            half[keep] = SCH_B
            maskB_d[r] = nc.inline_tensor(
                np.concatenate([half, half], axis=1), name=f"maskB{r}"
            )
    ones1_d = nc.inline_tensor(np.ones((1, 128), np.float16), name="ones1")

    Exp = mybir.ActivationFunctionType.Exp
    Ident = mybir.ActivationFunctionType.Identity
    Mult = mybir.AluOpType.mult
    Add = mybir.AluOpType.add

    with tile.TileContext(nc) as tc, ExitStack() as top:
        consts = top.enter_context(tc.tile_pool(name="consts", bufs=1))
        qk_pool = top.enter_context(tc.tile_pool(name="qkT", bufs=1))
        at_pool = top.enter_context(tc.tile_pool(name="attnT", bufs=1))
        wp_pool = top.enter_context(tc.tile_pool(name="wp", bufs=1))
        vb_pool = top.enter_context(tc.tile_pool(name="vb", bufs=1))
        xTp = top.enter_context(tc.tile_pool(name="xT", bufs=1))

        # ---- input DMAs -------------------------------------------------
        # spread over four queues (sync/gpsimd/scalar/vector), with wa[0],
        # xT[0], and the small constants first so the phase-A k-major sweep
        # and the v-bias broadcast are never DMA-gated for long
        xT = [xTp.tile([128, S], F16, tag=f"xT{k}", name=f"xT{k}") for k in range(NK)]
        wa_t = consts.tile([128, NK, CW], F16)
        # V with 64 ones columns appended: PSUM rows 64..127 of the PV
        # output all hold the softmax denominators (replicated). NOTE: the
        # ones memset must stay on gpsimd — the DVE InstMemset mis-lowers
        # this strided 4D region (wrong stride/count merge) and corrupts
        # the denominators.
        vb4 = vb_pool.tile([128, NSK, HPC, 128], F16)
        bvbc = consts.tile([128, HPC, 64], F32)

        nc.sync.dma_start(out=xT[0][:, 0:512], in_=x_d[0][:, 0:512])
        nc.sync.dma_start(out=xT[0][:, 512:S], in_=x_d[0][:, 512:S])
        for k in (2, 5, 6, 7):
            nc.sync.dma_start(out=xT[k][:], in_=x_d[k])
        for k in range(NK):
            nc.gpsimd.dma_start(out=wa_t[:, k, :], in_=wa_d[k])
        nc.gpsimd.memset(vb4[:, 0:4, :, 64:128], 1.0)
        ba_t = consts.tile([128, 4], F32)
        nc.scalar.dma_start(out=ba_t[:], in_=ba_d[0])
        ones1_t = consts.tile([1, 128], F16)
        nc.scalar.dma_start(out=ones1_t[:], in_=ones1_d.ap())
        bv_t = consts.tile([1, CP], F16)
        nc.scalar.dma_start(out=bv_t[:], in_=bv_d[0])
        for k in (1, 3, 4):
            nc.scalar.dma_start(out=xT[k][:], in_=x_d[k])
        wp_t = wp_pool.tile([128, 2, E], F16)
        for cc in range(2):
            nc.sync.dma_start(out=wp_t[:, cc, :], in_=wp_d[cc])
        mask01_t = consts.tile([128, 2, 2, 512], F16)
        for r in range(2):
            nc.scalar.dma_start(out=mask01_t[:, r, :, :], in_=mask01_d[r].ap())
        maskB_t = consts.tile([128, 2, 2, 512], F16)
        for r in range(2, 4):
            nc.gpsimd.dma_start(out=maskB_t[:, r - 2, :, :], in_=maskB_d[r].ap())
        nc.gpsimd.memset(vb4[:, 4:NSK, :, 64:128], 1.0)

        qkvT = [
            qk_pool.tile([128, S], F16, tag=f"qkT{cc}", name=f"qkT{cc}")
            for cc in range(4)
        ]
        attnT = [
            at_pool.tile([128, S], F16, tag=f"attnT{c}", name=f"attnT{c}")
            for c in range(2)
        ]

        with (
            tc.tile_pool(name="pp", bufs=18) as p_pool,
            tc.tile_pool(name="sm", bufs=2) as small,
            tc.tile_pool(name="ps_s", bufs=2, space="PSUM") as ps_s,
            tc.tile_pool(name="ps_pv", bufs=1, space="PSUM") as ps_pv,
            tc.tile_pool(name="ps_aux", bufs=2, space="PSUM") as ps_aux,
            tc.tile_pool(name="ob", bufs=2) as ob_pool,
        ):
            # ---- filler units: QKV/V work cut into PSUM-bank-lifetime
            # generators, pumped into attention-stream PE bubbles --------
            def qk_unit_gen(cc, rc):
                ps = ps_aux.tile([128, 512], F32, tag="aux", name="fq")
                for k in range(NK):
                    nc.tensor.matmul(
                        ps[:],
                        wa_t[:, k, cc * 128 : (cc + 1) * 128],
                        xT[k][:, rc * 512 : (rc + 1) * 512],
                        start=(k == 0),
                        stop=(k == NK - 1),
                    )
                    if k % 2 == 1 and k < NK - 1:
                        yield
                nc.scalar.activation(
                    qkvT[cc][:, rc * 512 : (rc + 1) * 512],
                    ps[:],
                    Ident,
                    bias=ba_t[:, cc : cc + 1],
                    scale=0.125 if cc < 2 else 1.0,
                )

            def v_unit_gen(pr, i):
                ps = ps_aux.tile([128, 512], F32, tag="aux", name="fv")
                for k in range(NK):
                    nc.tensor.matmul(
                        ps[:, 0:128],
                        xT[k][:, i * 128 : (i + 1) * 128],
                        wa_t[:, k, 512 + pr * 128 : 640 + pr * 128],
                        start=(k == 0),
                        stop=(k == NK - 1),
                    )
                for h in range(2):
                    nc.vector.tensor_add(
                        vb4[:, i, 2 * pr + h, 0:64],
                        ps[:, h * 64 : h * 64 + 64],
                        bvbc[:, 2 * pr + h, :],
                    )
                yield

            # work queue: (deadline_group, generator, started); deadlines are
            # one group EARLY (evacuations queue behind a whole group of
            # exps, so a unit finished exactly at its reader's boundary
            # would stall the reader ~5us on the evac)
            work = []
            for g in (1, 2, 3):
                d = max(1, g - 1)
                work.append([d, qk_unit_gen(0, g), False])
                work.append([d, qk_unit_gen(2, g), False])
                for i in range(4 * g, 4 * g + 4):
                    work.append([d, v_unit_gen(0, i), False])
            for g in (4, 5, 6, 7):
                J = g - 4
                work.append([g - 1, qk_unit_gen(1, J), False])
                work.append([g - 1, qk_unit_gen(3, J), False])
                for i in range(4 * J, 4 * J + 4):
                    work.append([g - 1, v_unit_gen(1, i), False])

            def pump(n):
                for _ in range(n):
                    if not work:
                        return
                    ent = work[0]
                    ent[2] = True
                    try:
                        next(ent[1])
                    except StopIteration:
                        work.pop(0)

            def finish_inflight():
                # run a suspended unit to completion so its PSUM ring slot
                # is safe to rotate past (emit_proj allocates from the ring)
                if work and work[0][2]:
                    for _ in work[0][1]:
                        pass
                    work.pop(0)

            def drain_until(G):
                while work and work[0][0] <= G:
                    for _ in work[0][1]:
                        pass
                    work.pop(0)

            # ---- phase A: pair-0 prerequisites up front ----------------
            # the q/k rc=0 accumulations run k-major (two interleaved
            # accumulators, same pattern as the classic 4-way rc loop) so
            # each arriving xT chunk is consumed by two matmuls at once;
            # the v-bias broadcast borrows the (idle until G0) pvp tile
            # rather than an aux ring slot
            psb = ps_pv.tile([128, 2, 512], F32, tag="pv", name="pvb")
            nc.tensor.matmul(
                psb[:, 0, 0:CP], ones1_t[0:1, :], bv_t[0:1, :],
                start=True, stop=True,
            )
            nc.vector.tensor_copy(bvbc[:].opt(), psb[:, 0, 0:CP])
            qk00 = ps_aux.tile([128, 512], F32, tag="aux", name="fq")
            qk20 = ps_aux.tile([128, 512], F32, tag="aux", name="fq")
            for k in range(NK):
                st = {"start": k == 0, "stop": k == NK - 1}
                nc.tensor.matmul(
                    qk00[:], wa_t[:, k, 0:128], xT[k][:, 0:512], **st
                )
                nc.tensor.matmul(
                    qk20[:], wa_t[:, k, 256:384], xT[k][:, 0:512], **st
                )
            nc.scalar.activation(
                qkvT[0][:, 0:512], qk00[:], Ident,
                bias=ba_t[:, 0:1], scale=0.125,
            )
            nc.scalar.activation(
                qkvT[2][:, 0:512], qk20[:], Ident,
                bias=ba_t[:, 2:3], scale=1.0,
            )
            for i in range(4):
                for _ in v_unit_gen(0, i):
                    pass

            act_t = 0.0
            dve_t = 0.0
            last_eng = []

            def emit_proj(Jq):
                ob = ob_pool.tile([128, 8, 512], F16, tag="ob", name="ob")
                for eo in range(8):
                    pp = ps_aux.tile([128, 512], F32, tag="aux", name="prj")
                    for cc in range(2):
                        nc.tensor.matmul(
                            pp[:],
                            wp_t[:, cc, eo * 128 : (eo + 1) * 128],
                            attnT[cc][:, Jq * 512 : (Jq + 1) * 512],
                            start=(cc == 0),
                            stop=(cc == 1),
                        )
                    if eo % 2 == 0:
                        nc.scalar.copy(ob[:, eo, :], pp[:])
                    else:
                        nc.vector.tensor_copy(ob[:, eo, :], pp[:])
                    if eo == 3:
                        nc.sync.dma_start(
                            out=out_d[:, 0:4, Jq * 512 : (Jq + 1) * 512],
                            in_=ob[:, 0:4, :],
                        )
                nc.sync.dma_start(
                    out=out_d[:, 4:8, Jq * 512 : (Jq + 1) * 512],
                    in_=ob[:, 4:8, :],
                )

            pending = []
            for G in range(8):
                pr, J = G // 4, G % 4
                drain_until(G)
                qT = qkvT[pr]
                kT = qkvT[2 + pr]
                nblk = 4 * J + 4
                sq = bass.ts(J, 512)
                order = [4 * J + r for r in range(4)] + list(range(4 * J))
                pblks = {}
                nissued = 0
                for i in order:
                    r = i - 4 * J
                    w = 512 if r < 0 else 512 - 128 * r
                    sqo = J * 512 + (0 if r < 0 else 128 * r)
                    stile = ps_s.tile([128, 2, 512], F32, tag="s", name="sps")
                    for hh in range(2):
                        o = hh * 64
                        nc.tensor.matmul(
                            stile[:, hh, 0:w],
                            kT[o : o + 64, i * 128 : (i + 1) * 128],
                            qT[o : o + 64, sqo : sqo + w],
                            start=True,
                            stop=True,
                            tile_position=(o, 0),
                        )
                    pt = p_pool.tile([128, 2, 512], U16, tag="p", name="p")
                    if r >= 0 and r < 2:
                        act_t += ACT_COST(2 * w)
                        if r == 0:
                            nc.scalar.activation(
                                pt[:].opt().bitcast(F16), stile[:].opt(), Exp
                            )
                            dve_t += DVE_COST(1024) * 0.5
                            nc.vector.tensor_mul(
                                pt[:].opt().bitcast(F16),
                                pt[:].opt().bitcast(F16),
                                mask01_t[:, r, :, :].opt(),
                            )
                        else:
                            nc.scalar.activation(
                                pt[:, :, 0:w].bitcast(F16), stile[:, :, 0:w], Exp
                            )
                            dve_t += DVE_COST(2 * w) * 0.5
                            nc.vector.tensor_mul(
                                pt[:, :, 0:w].bitcast(F16),
                                pt[:, :, 0:w].bitcast(F16),
                                mask01_t[:, r, :, 0:w],
                            )
                    elif r >= 2:
                        dve_t += DVE_COST(2 * w)
                        nc.vector.scalar_tensor_tensor(
                            out=pt[:, :, 0:w],
                            in0=stile[:, :, 0:w],
                            scalar=SCH_A,
                            in1=maskB_t[:, r - 2, :, 0:w],
                            op0=Mult,
                            op1=Add,
                        )
                    else:
                        ca, cd = ACT_COST(1024), DVE_COST(1024)
                        use_act = act_t + ca <= dve_t + cd
                        if len(last_eng) >= 2 and last_eng[-1] == last_eng[-2]:
                            use_act = not last_eng[-1]
                        last_eng.append(use_act)
                        if use_act:
                            act_t += ca
                            nc.scalar.activation(
                                pt[:].opt().bitcast(F16), stile[:].opt(), Exp
                            )
                        else:
                            dve_t += cd
                            nc.vector.tensor_scalar(
                                pt[:].opt(), stile[:].opt(), SCH_A, SCH_B,
                                Mult, Add,
                            )
                    pblks[i] = pt
                    nissued += 1
                    pump(1)
                    # the previous chunk's projection lands a few blocks
                    # into this stream, after its attnT inputs are written
                    if nissued == 3 and pending:
                        finish_inflight()
                        for fn in pending:
                            fn()
                        pending = []
                if pending:
                    finish_inflight()
                    for fn in pending:
                        fn()
                    pending = []

                last = G == 7
                pvp = ps_pv.tile([128, 2, 512], F32, tag="pv", name="pvp")
                dn = small.tile([128, 1024], F16, tag="dn", name="dn")
                y0 = small.tile([128, 1024], I16, tag="y0", name="y0")
                t1 = small.tile([128, 1024], F16, tag="t1", name="t1")
                for hh in range(2):
                    hs = slice(hh * 512, (hh + 1) * 512)
                    for n, i in enumerate(order):
                        r = i - 4 * J
                        w = 512 if r < 0 else 512 - 128 * r
                        co = 0 if r < 0 else 128 * r
                        nc.tensor.matmul(
                            pvp[:, hh, co : co + w],
                            vb4[:, i, 2 * pr + hh, :],
                            pblks[i][:, hh, 0:w].bitcast(F16),
                            start=(n == 0),
                            stop=(n == nblk - 1),
                        )
                    act_t += ACT_COST(512)
                    nc.scalar.copy(dn[0:64, hs], pvp[64:128, hh, :])
                    if last:
                        # tail: run the whole reciprocal + normalize chain
                        # per head so head 0's chain hides under head 1's
                        # PV burst
                        nc.vector.tensor_scalar(
                            y0[0:64, hs], dn[0:64, hs].bitcast(I16),
                            -1.0, RCP_K, Mult, Add,
                        )
                        nc.vector.scalar_tensor_tensor(
                            out=t1[0:64, hs], in0=dn[0:64, hs], scalar=-1.0,
                            in1=y0[0:64, hs].bitcast(F16), op0=Mult, op1=Mult,
                        )
                        nc.vector.scalar_tensor_tensor(
                            out=t1[0:64, hs], in0=t1[0:64, hs], scalar=2.0,
                            in1=y0[0:64, hs].bitcast(F16), op0=Add, op1=Mult,
                        )
                        nc.vector.tensor_mul(
                            attnT[pr][hh * 64 : hh * 64 + 64, sq],
                            pvp[0:64, hh, :],
                            t1[0:64, hs],
                        )
                if not last:
                    nc.vector.tensor_scalar(
                        y0[0:64, :], dn[0:64, :].bitcast(I16),
                        -1.0, RCP_K, Mult, Add,
                    )
                    nc.vector.scalar_tensor_tensor(
                        out=t1[0:64, :], in0=dn[0:64, :], scalar=-1.0,
                        in1=y0[0:64, :].bitcast(F16), op0=Mult, op1=Mult,
                    )
                    nc.vector.scalar_tensor_tensor(
                        out=t1[0:64, :], in0=t1[0:64, :], scalar=2.0,
                        in1=y0[0:64, :].bitcast(F16), op0=Add, op1=Mult,
                    )
                    for hh in range(2):
                        nc.vector.tensor_mul(
                            attnT[pr][hh * 64 : hh * 64 + 64, sq],
                            pvp[0:64, hh, :],
                            t1[0:64, hh * 512 : (hh + 1) * 512],
                        )
                dve_t += DVE_NORM_EXTRA

                if pr == 1:
                    if last:
                        emit_proj(J)
                    else:
                        pending.append(lambda Jq=J: emit_proj(Jq))

    nc.compile()
    return nc


def _col_perm(g):
    """Per-core W_attn column permutation: [q0..q3 | k0..k3 | v0..v3]."""
    cols = []
    for t in range(3):          # q, k, v
        for h in range(HPC):
            base = (4 * g + h) * 3 * HD + t * HD
            cols.append(np.arange(base, base + HD))
    return np.concatenate(cols)


def kernel(hidden_states, W_attn, b_attn, W_proj, b_proj):
    hidden_states = np.asarray(hidden_states, np.float32)
    W_attn = np.asarray(W_attn, np.float32)
    b_attn = np.asarray(b_attn, np.float32)
    W_proj = np.asarray(W_proj, np.float32)
    b_proj = np.asarray(b_proj, np.float32)

    if "nc" not in _cache:
        _cache["nc"] = _build()
    nc = _cache["nc"]

    # q columns (first 256 of the permuted layout) have scale 1/8 folded into
    # the PSUM->SBUF copy; bias is added after the scale, so pre-scale it.
    bias_scale = np.ones(2 * CP, np.float32)
    bias_scale[:CP] = 0.125

    in_maps = []
    for c in range(8):
        b, g = divmod(c, 4)
        perm = _col_perm(g)
        wa = np.ascontiguousarray(W_attn[:, perm])
        ba = np.ascontiguousarray(
            (b_attn[perm][: 2 * CP] * bias_scale).astype(np.float32).reshape(4, 128).T
        )
        bv = b_attn[perm][2 * CP :].astype(np.float16)
        wp = np.ascontiguousarray(W_proj[g * CP : (g + 1) * CP, :])
        xT = np.ascontiguousarray(hidden_states[b].T).astype(np.float16)
        in_maps.append(
            {
                "x": xT.reshape(NK, 128, S),
                "wa": wa.astype(np.float16).reshape(NK, 128, CW),
                "ba": ba.reshape(1, 128, 4),
                "bv": bv.reshape(1, 1, CP),
                "wp": wp.astype(np.float16).reshape(2, 128, E),
            }
        )

    global _last_in_maps
    _last_in_maps = in_maps
    res = run_bass_kernel_spmd(nc, in_maps, list(range(8)))

    out = np.zeros((B, S, E), np.float32)
    for c in range(8):
        b = c // 4
        # out_t is [128, 8, S]; output feature e = eo*128 + p
        ot = res.results[c]["out_t"].transpose(1, 0, 2).reshape(E, S)
        out[b] += ot.astype(np.float32).T
    out += b_proj
    return out
